# revision 21
# baseline (speedup 1.0000x reference)
"""Trainium2 Bass kernel for 16-head MHA (B=8, K=1024, D=1024, head_dim=64).

Sharding: pure data-parallel over batch - core c computes batch element c.
No collectives needed (B == n_cores == 8).

Per-core algorithm (layouts chosen so no big transposes are needed):
  - X^T [e, i] tiles via hardware DMA transpose (fp16).
  - qT[hd, i] = Wq[e,hd].T-slices @ X^T   (+ per-partition bias)   [same kT]
  - v [j, hd'] = X^T-slices @ Wv_aug      (+ broadcast bias; Wv_aug has a
    zero column per head whose bias is 1.0 -> constant ones column in v)
  - scores^T tile [j,i] = kT-slice.T @ qT-slice, two heads row-packed on PE
    row groups 0-63/64-127 into one 2-bank PSUM tile, one exp (ACT) over
    both -> E^T (fp16)
  - attn@V: out^T[d,i] (+denominator row 64, from the ones column of v)
    accumulated over j blocks; denominator -> partition 0 via tiny DMA,
    gpsimd partition_broadcast + reciprocal_approx_fast + DVE multiply.
  - out-projection: final[i,e] = O^T-slices.T @ Wo-slices (+ bias).

The q-projection and the output projection are interleaved into the
attention phase as tensor-engine filler (the attention phase is
ACT(exp)-bound), keeping the PE dense so the HAM clock gate stays open.
"""

import os
import sys
from contextlib import ExitStack

import numpy as np

if "/opt/trn_rl_repo" not in sys.path:
    sys.path.insert(0, "/opt/trn_rl_repo")

import concourse.bacc as bacc
import concourse.mybir as mybir
import concourse.tile as tile
from concourse.bass_utils import run_bass_kernel_spmd

F32 = mybir.dt.float32
F32R = mybir.dt.float32r
F16 = mybir.dt.float16

B = 8          # batch == number of cores
SEQ = 1024     # sequence length
D = 1024       # d_model
H = 16         # heads
DH = 64        # head dim
P = 128        # partitions
NB = SEQ // P  # 8 row blocks
IC = 512       # i-chunk (moving free dim)
NIC = SEQ // IC
NE = D // P    # 8 contraction slices
VW = H * (DH + 1)  # 1040: v with a ones column per head
SCALE = 1.0 / np.sqrt(np.float32(SEQ))

Exp = mybir.ActivationFunctionType.Exp


def _build():
    nc = bacc.Bacc(
        "TRN2",
        target_bir_lowering=False,
        debug=False,
        enable_asserts=False,
        num_devices=B,
    )

    xq = nc.dram_tensor("xq", [SEQ, D], F16, kind="ExternalInput").ap()
    xk = nc.dram_tensor("xk", [SEQ, D], F16, kind="ExternalInput").ap()
    xv = nc.dram_tensor("xv", [SEQ, D], F16, kind="ExternalInput").ap()
    wq = nc.dram_tensor("wq", [D, D], F16, kind="ExternalInput").ap()
    wk = nc.dram_tensor("wk", [D, D], F16, kind="ExternalInput").ap()
    wva = nc.dram_tensor("wva", [D, VW], F16, kind="ExternalInput").ap()
    wo = nc.dram_tensor("wo", [D, D], F16, kind="ExternalInput").ap()
    bq2 = nc.dram_tensor("bq2", [P, NE], F32, kind="ExternalInput").ap()
    bk2 = nc.dram_tensor("bk2", [P, NE], F32, kind="ExternalInput").ap()
    bvb = nc.dram_tensor("bvb", [P, VW], F32, kind="ExternalInput").ap()
    bob = nc.dram_tensor("bob", [P, D], F32, kind="ExternalInput").ap()
    out = nc.dram_tensor("out", [SEQ, D], F32, kind="ExternalOutput").ap()

    with tile.TileContext(nc, pool_alloc_mode="queue") as tc, ExitStack() as st:
        pers = st.enter_context(tc.tile_pool(name="pers", bufs=1))

        bq_sb = pers.tile([P, NE], F32, tag="bq_sb")
        nc.sync.dma_start(bq_sb[:], bq2[:])
        bk_sb = pers.tile([P, NE], F32, tag="bk_sb")
        nc.sync.dma_start(bk_sb[:], bk2[:])
        bv_sb = pers.tile([P, VW], F32, tag="bv_sb")
        nc.sync.dma_start(bv_sb[:], bvb[:])
        bo_sb = pers.tile([P, D], F32, tag="bo_sb")
        nc.sync.dma_start(bo_sb[:], bob[:])

        # PE warm-up: dense dummy matmuls so the HAM clock gate opens
        # before the first real matmuls (DMA transposes don't touch PE).
        with (
            tc.tile_pool(name="wup", bufs=1) as wup,
            tc.tile_pool(name="psum_wu", bufs=2, space="PSUM") as psum_wu,
        ):
            dm = wup.tile([P, IC], F16, tag="dm")
            nc.vector.memset(dm[:], 0.125)
            for i in range(52):
                pw = psum_wu.tile([P, IC], F32, tag="wu", name="pw")
                nc.tensor.matmul(
                    pw[:], dm[:, 0:P], dm[:], start=True, stop=True
                )

        qT = [pers.tile([P, SEQ], F16, tag=f"qT{i}", name=f"qT{i}") for i in range(NE)]
        kT = [pers.tile([P, SEQ], F16, tag=f"kT{i}", name=f"kT{i}") for i in range(NE)]
        vv = [pers.tile([P, VW], F16, tag=f"vv{i}", name=f"vv{i}") for i in range(NB)]
        OT = [pers.tile([P, SEQ], F16, tag=f"OT{i}", name=f"OT{i}") for i in range(NE)]

        # attention-phase pools (open first so the q-projection pools
        # below can be released mid-kernel in LIFO order)
        etp = st.enter_context(tc.tile_pool(name="etp", bufs=12))
        oup = st.enter_context(tc.tile_pool(name="oup", bufs=3))
        dnp = st.enter_context(tc.tile_pool(name="dnp", bufs=3))
        rbp = st.enter_context(tc.tile_pool(name="rbp", bufs=4))
        stp = st.enter_context(tc.tile_pool(name="stp", bufs=2))
        fop = st.enter_context(tc.tile_pool(name="fop", bufs=3))

        # pools for the q-projection live through phase A and the first
        # half of the attention phase (freed mid-kernel)
        stA = ExitStack()
        wpool = stA.enter_context(tc.tile_pool(name="wpool", bufs=16))
        xtp = stA.enter_context(tc.tile_pool(name="xtp", bufs=16))

        def transposed_input(x_dram, names):
            """X^T [e, i] tiles via hardware DMA transpose (fp16)."""
            xt = []
            for eb in range(NE):
                x_sb = xtp.tile([P, SEQ], F16, tag="xt", name=f"{names}{eb}")
                nc.sync.dma_start_transpose(
                    x_sb[:], x_dram[:, eb * P : (eb + 1) * P]
                )
                xt.append(x_sb)
            return xt

        def load_w(w_dram, ncols, names):
            wt = []
            for eb in range(NE):
                w_sb = wpool.tile([P, VW], F16, tag="wt", name=f"{names}{eb}")
                nc.sync.dma_start(
                    w_sb[:, :ncols], w_dram[eb * P : (eb + 1) * P, :]
                )
                wt.append(w_sb)
            return wt

        # ---- phase A: v and k projections -------------------------------
        with (
            tc.tile_pool(name="psum_big", bufs=6, space="PSUM") as psum_big,
            tc.tile_pool(name="psum_vt", bufs=2, space="PSUM") as psum_vt,
        ):
            wvt = load_w(wva, VW, "wv")
            xtv = transposed_input(xv, "xv")
            VCH = [(0, 512), (512, 512), (1024, VW - 1024)]
            for jb in range(NB):
                for coff, clen in VCH:
                    if clen == 512:
                        pv = psum_big.tile([P, clen], F32, tag="big", name="pv")
                    else:
                        pv = psum_vt.tile([P, clen], F32, tag="vt", name="pv")
                    for eb in range(NE):
                        nc.tensor.matmul(
                            pv[:],
                            xtv[eb][:, jb * P : (jb + 1) * P],
                            wvt[eb][:, coff : coff + clen],
                            start=(eb == 0),
                            stop=(eb == NE - 1),
                        )
                    nc.vector.tensor_add(
                        vv[jb][:, coff : coff + clen],
                        pv[:],
                        bv_sb[:, coff : coff + clen],
                    )

            wkt = load_w(wk, D, "wk")
            xtk = transposed_input(xk, "xk")
            for hb in range(NE):
                for ic in range(NIC):
                    pq = psum_big.tile([P, IC], F32, tag="big", name="pq")
                    for eb in range(NE):
                        nc.tensor.matmul(
                            pq[:],
                            wkt[eb][:, hb * P : (hb + 1) * P],
                            xtk[eb][:, ic * IC : (ic + 1) * IC],
                            start=(eb == 0),
                            stop=(eb == NE - 1),
                        )
                    nc.vector.tensor_scalar_add(
                        kT[hb][:, ic * IC : (ic + 1) * IC],
                        pq[:],
                        bk_sb[:, hb : hb + 1],
                    )

            # q-projection resources: prefetch during phase A
            wqt = load_w(wq, D, "wq")
            xtq = transposed_input(xq, "xq")

        # ---- phases B+C: attention + q-proj + output projection ---------
        with (
            tc.tile_pool(name="psum_sc", bufs=3, space="PSUM") as psum_sc,
            tc.tile_pool(name="psum_ov", bufs=2, space="PSUM") as psum_ov,
        ):
            def emit_qproj(hb):
                for ic in range(NIC):
                    pq2 = psum_sc.tile([P, 2 * IC], F32, tag="sc", name="pq2")
                    pq = pq2[:, 0:IC]
                    for eb in range(NE):
                        nc.tensor.matmul(
                            pq,
                            wqt[eb][:, hb * P : (hb + 1) * P],
                            xtq[eb][:, ic * IC : (ic + 1) * IC],
                            start=(eb == 0),
                            stop=(eb == NE - 1),
                        )
                    nc.vector.tensor_scalar_add(
                        qT[hb][:, ic * IC : (ic + 1) * IC],
                        pq,
                        bq_sb[:, hb : hb + 1],
                    )

            rounds = [(ic, t) for ic in range(NIC) for t in range(NE)]
            et_store = {}

            def emit_scores(r):
                ic, t = rounds[r]
                icsl = slice(ic * IC, (ic + 1) * IC)
                et = {}
                for jb in range(NB):
                    ps = psum_sc.tile([P, 2 * IC], F32, tag="sc", name="ps")
                    for hh in range(2):
                        off = 64 * hh
                        nc.tensor.matmul(
                            ps[:, hh * IC : (hh + 1) * IC],
                            kT[t][off : off + DH, jb * P : (jb + 1) * P],
                            qT[t][off : off + DH, icsl],
                            start=True,
                            stop=True,
                            tile_position=(off, 0),
                        )
                    e = etp.tile([P, 2 * IC], F16, tag="et", name="et")
                    nc.scalar.activation(e[:], ps[:], Exp, scale=float(SCALE))
                    et[jb] = e
                et_store[r] = et

            def emit_attnv(r):
                ic, t = rounds[r]
                icsl = slice(ic * IC, (ic + 1) * IC)
                et = et_store.pop(r)
                for hh in range(2):
                    h = 2 * t + hh
                    po = psum_ov.tile([DH + 1, IC], F32, tag="ov", name="po")
                    for jb in range(NB):
                        nc.tensor.matmul(
                            po[:],
                            vv[jb][:, h * (DH + 1) : (h + 1) * (DH + 1)],
                            et[jb][:, hh * IC : (hh + 1) * IC],
                            start=(jb == 0),
                            stop=(jb == NB - 1),
                        )
                    # copy PSUM out early (incl. denominator row) so the
                    # 2-bank ov pool isn't held through the recip chain
                    ou = oup.tile([DH + 1, IC], F32, tag="ou", name="ou")
                    nc.vector.tensor_copy(ou[:], po[:])
                    d0 = dnp.tile([1, IC], F32, tag="d0", name="d0")
                    nc.sync.dma_start(d0[0:1, :], ou[DH : DH + 1, :])
                    dn = rbp.tile([DH, IC], F32, tag="dn", name="dn")
                    nc.gpsimd.partition_broadcast(
                        dn[:], d0[0:1, :], channels=DH
                    )
                    rb = rbp.tile([DH, IC], F32, tag="rb", name="rb")
                    nc.vector.reciprocal_approx_fast(rb[:], dn[:])
                    if hh == 0:
                        nc.vector.tensor_mul(
                            OT[t][0:DH, icsl], ou[0:DH, :], rb[:]
                        )
                    else:
                        sg = stp.tile([DH, IC], F16, tag="sg", name="sg")
                        nc.vector.tensor_mul(sg[:], ou[0:DH, :], rb[:])
                        nc.sync.dma_start(OT[t][DH:P, icsl], sg[:])

            nr = len(rounds)
            # q-projection interleaved ahead of the score rounds that use it
            emit_qproj(0)
            emit_qproj(1)
            emit_scores(0)
            for r in range(NE - 2):
                emit_qproj(r + 2)
                emit_scores(r + 1)
                emit_attnv(r)
            # q-projection fully emitted -> free its pools, load Wo
            stA.close()
            with tc.tile_pool(name="wop", bufs=8) as wop:
                wot = []
                for hb in range(NE):
                    wo_sb = wop.tile([P, D], F16, tag="wo", name=f"wo{hb}")
                    nc.sync.dma_start(wo_sb[:], wo[hb * P : (hb + 1) * P, :])
                    wot.append(wo_sb)

                def emit_outproj(ib, ec):
                    pf2 = psum_sc.tile([P, 2 * IC], F32, tag="sc", name="pf2")
                    pf = pf2[:, 0:IC]
                    for hb in range(NE):
                        nc.tensor.matmul(
                            pf,
                            OT[hb][:, ib * P : (ib + 1) * P],
                            wot[hb][:, ec * IC : (ec + 1) * IC],
                            start=(hb == 0),
                            stop=(hb == NE - 1),
                        )
                    fo = fop.tile([P, IC], F32, tag="fo", name="fo")
                    nc.vector.tensor_add(
                        fo[:], pf[:], bo_sb[:, ec * IC : (ec + 1) * IC]
                    )
                    nc.sync.dma_start(
                        out[ib * P : (ib + 1) * P, ec * IC : (ec + 1) * IC],
                        fo[:],
                    )

                fillers = [
                    (ib, ec) for ib in range(NB // 2) for ec in range(NIC)
                ]
                late = [
                    (ib, ec) for ib in range(NB // 2, NB) for ec in range(NIC)
                ]
                for r in range(NE - 2, nr):
                    if r + 1 < nr:
                        emit_scores(r + 1)
                    emit_attnv(r)
                    if r >= NE and fillers:
                        emit_outproj(*fillers.pop(0))
                for ib, ec in fillers + late:
                    emit_outproj(ib, ec)

    nc.compile()
    return nc


_NC = None


def _get_nc():
    global _NC
    if _NC is None:
        _NC = _build()
    return _NC


def _prep_inputs(query, key, value, Wq, bq, Wk, bk, Wv, bv, Wo, bo):
    f32 = lambda a: np.ascontiguousarray(np.asarray(a, dtype=np.float32))
    f16 = lambda a: np.ascontiguousarray(
        np.asarray(a, dtype=np.float32).astype(np.float16)
    )
    query, key, value = f16(query), f16(key), f16(value)
    bq, bk, bv, bo = f32(bq), f32(bk), f32(bv), f32(bo)
    Wv = f32(Wv)

    wva = np.zeros((D, VW), dtype=np.float32)
    bva = np.zeros((VW,), dtype=np.float32)
    for h in range(H):
        wva[:, h * (DH + 1) : h * (DH + 1) + DH] = Wv[:, h * DH : (h + 1) * DH]
        bva[h * (DH + 1) : h * (DH + 1) + DH] = bv[h * DH : (h + 1) * DH]
        bva[h * (DH + 1) + DH] = 1.0
    common = {
        "wq": f16(Wq),
        "wk": f16(Wk),
        "wva": wva.astype(np.float16),
        "wo": f16(Wo),
        "bq2": np.ascontiguousarray(bq.reshape(NE, P).T),
        "bk2": np.ascontiguousarray(bk.reshape(NE, P).T),
        "bvb": np.ascontiguousarray(np.tile(bva, (P, 1))),
        "bob": np.ascontiguousarray(np.tile(bo, (P, 1))),
    }
    in_maps = [
        dict(common, xq=query[c], xk=key[c], xv=value[c]) for c in range(B)
    ]
    return in_maps


def kernel(query, key, value, Wq, bq, Wk, bk, Wv, bv, Wo, bo):
    nc = _get_nc()
    in_maps = _prep_inputs(query, key, value, Wq, bq, Wk, bk, Wv, bv, Wo, bo)
    res = run_bass_kernel_spmd(nc, in_maps, core_ids=list(range(B)))
    return np.stack([res.results[c]["out"] for c in range(B)], axis=0)


# revision 22
# speedup vs baseline: 1.0260x; 1.0260x over previous
"""Trainium2 Bass kernel for 16-head MHA (B=8, K=1024, D=1024, head_dim=64).

Sharding: pure data-parallel over batch - core c computes batch element c.
No collectives needed (B == n_cores == 8).

Per-core algorithm (layouts chosen so no big transposes are needed):
  - X^T [e, i] tiles via hardware DMA transpose (fp16).
  - qT[hd, i] = Wq[e,hd].T-slices @ X^T   (+ per-partition bias)   [same kT]
  - v [j, hd'] = X^T-slices @ Wv_aug      (+ broadcast bias; Wv_aug has a
    zero column per head whose bias is 1.0 -> constant ones column in v)
  - scores^T tile [j,i] = kT-slice.T @ qT-slice, two heads row-packed on PE
    row groups 0-63/64-127 into one 2-bank PSUM tile, one exp (ACT) over
    both -> E^T (fp16)
  - attn@V: out^T[d,i] (+denominator row 64, from the ones column of v)
    accumulated over j blocks; denominator -> partition 0 via tiny DMA,
    gpsimd partition_broadcast + reciprocal_approx_fast + DVE multiply.
  - out-projection: final[i,e] = O^T-slices.T @ Wo-slices (+ bias).

The q-projection and the output projection are interleaved into the
attention phase as tensor-engine filler (the attention phase is
ACT(exp)-bound), keeping the PE dense so the HAM clock gate stays open.
"""

import os
import sys
from contextlib import ExitStack

import numpy as np

if "/opt/trn_rl_repo" not in sys.path:
    sys.path.insert(0, "/opt/trn_rl_repo")

import concourse.bacc as bacc
import concourse.mybir as mybir
import concourse.tile as tile
from concourse.bass_utils import run_bass_kernel_spmd

F32 = mybir.dt.float32
F32R = mybir.dt.float32r
F16 = mybir.dt.float16

B = 8          # batch == number of cores
SEQ = 1024     # sequence length
D = 1024       # d_model
H = 16         # heads
DH = 64        # head dim
P = 128        # partitions
NB = SEQ // P  # 8 row blocks
IC = 512       # i-chunk (moving free dim)
NIC = SEQ // IC
NE = D // P    # 8 contraction slices
VW = H * (DH + 1)  # 1040: v with a ones column per head
SCALE = 1.0 / np.sqrt(np.float32(SEQ))

Exp = mybir.ActivationFunctionType.Exp


def _build():
    nc = bacc.Bacc(
        "TRN2",
        target_bir_lowering=False,
        debug=False,
        enable_asserts=False,
        num_devices=B,
    )

    xq = nc.dram_tensor("xq", [SEQ, D], F16, kind="ExternalInput").ap()
    xk = nc.dram_tensor("xk", [SEQ, D], F16, kind="ExternalInput").ap()
    xv = nc.dram_tensor("xv", [SEQ, D], F16, kind="ExternalInput").ap()
    wq = nc.dram_tensor("wq", [D, D], F16, kind="ExternalInput").ap()
    wk = nc.dram_tensor("wk", [D, D], F16, kind="ExternalInput").ap()
    wva = nc.dram_tensor("wva", [D, VW], F16, kind="ExternalInput").ap()
    wo = nc.dram_tensor("wo", [D, D], F16, kind="ExternalInput").ap()
    bq2 = nc.dram_tensor("bq2", [P, NE], F32, kind="ExternalInput").ap()
    bk2 = nc.dram_tensor("bk2", [P, NE], F32, kind="ExternalInput").ap()
    bvb = nc.dram_tensor("bvb", [P, VW], F32, kind="ExternalInput").ap()
    bob = nc.dram_tensor("bob", [P, D], F32, kind="ExternalInput").ap()
    out = nc.dram_tensor("out", [SEQ, D], F32, kind="ExternalOutput").ap()

    with tile.TileContext(nc, pool_alloc_mode="queue") as tc, ExitStack() as st:
        pers = st.enter_context(tc.tile_pool(name="pers", bufs=1))

        bq_sb = pers.tile([P, NE], F32, tag="bq_sb")
        nc.sync.dma_start(bq_sb[:], bq2[:])
        bk_sb = pers.tile([P, NE], F32, tag="bk_sb")
        nc.sync.dma_start(bk_sb[:], bk2[:])
        bv_sb = pers.tile([P, VW], F32, tag="bv_sb")
        nc.sync.dma_start(bv_sb[:], bvb[:])
        bo_sb = pers.tile([P, D], F32, tag="bo_sb")
        nc.sync.dma_start(bo_sb[:], bob[:])

        # PE warm-up: dense dummy matmuls so the HAM clock gate opens
        # before the first real matmuls (DMA transposes don't touch PE).
        with (
            tc.tile_pool(name="wup", bufs=1) as wup,
            tc.tile_pool(name="psum_wu", bufs=2, space="PSUM") as psum_wu,
        ):
            dm = wup.tile([P, IC], F16, tag="dm")
            nc.vector.memset(dm[:], 0.125)
            for i in range(40):
                pw = psum_wu.tile([P, IC], F32, tag="wu", name="pw")
                nc.tensor.matmul(
                    pw[:], dm[:, 0:P], dm[:], start=True, stop=True
                )

        qT = [pers.tile([P, SEQ], F16, tag=f"qT{i}", name=f"qT{i}") for i in range(NE)]
        kT = [pers.tile([P, SEQ], F16, tag=f"kT{i}", name=f"kT{i}") for i in range(NE)]
        vv = [pers.tile([P, VW], F16, tag=f"vv{i}", name=f"vv{i}") for i in range(NB)]
        OT = [pers.tile([P, SEQ], F16, tag=f"OT{i}", name=f"OT{i}") for i in range(NE)]

        # attention-phase pools (open first so the q-projection pools
        # below can be released mid-kernel in LIFO order)
        etp = st.enter_context(tc.tile_pool(name="etp", bufs=12))
        oup = st.enter_context(tc.tile_pool(name="oup", bufs=3))
        dnp = st.enter_context(tc.tile_pool(name="dnp", bufs=3))
        rbp = st.enter_context(tc.tile_pool(name="rbp", bufs=4))
        stp = st.enter_context(tc.tile_pool(name="stp", bufs=2))
        fop = st.enter_context(tc.tile_pool(name="fop", bufs=3))

        # pools for the q-projection live through phase A and the first
        # half of the attention phase (freed mid-kernel)
        stA = ExitStack()
        wpool = stA.enter_context(tc.tile_pool(name="wpool", bufs=16))
        xtp = stA.enter_context(tc.tile_pool(name="xtp", bufs=16))

        def transposed_input(x_dram, names):
            """X^T [e, i] tiles via hardware DMA transpose (fp16)."""
            xt = []
            for eb in range(NE):
                x_sb = xtp.tile([P, SEQ], F16, tag="xt", name=f"{names}{eb}")
                nc.sync.dma_start_transpose(
                    x_sb[:], x_dram[:, eb * P : (eb + 1) * P]
                )
                xt.append(x_sb)
            return xt

        def load_w(w_dram, ncols, names):
            wt = []
            for eb in range(NE):
                w_sb = wpool.tile([P, VW], F16, tag="wt", name=f"{names}{eb}")
                nc.sync.dma_start(
                    w_sb[:, :ncols], w_dram[eb * P : (eb + 1) * P, :]
                )
                wt.append(w_sb)
            return wt

        # ---- phase A: v and k projections -------------------------------
        with (
            tc.tile_pool(name="psum_big", bufs=6, space="PSUM") as psum_big,
            tc.tile_pool(name="psum_vt", bufs=2, space="PSUM") as psum_vt,
        ):
            wvt = load_w(wva, VW, "wv")
            xtv = transposed_input(xv, "xv")
            VCH = [(0, 512), (512, 512), (1024, VW - 1024)]
            for jb in range(NB):
                for coff, clen in VCH:
                    if clen == 512:
                        pv = psum_big.tile([P, clen], F32, tag="big", name="pv")
                    else:
                        pv = psum_vt.tile([P, clen], F32, tag="vt", name="pv")
                    for eb in range(NE):
                        nc.tensor.matmul(
                            pv[:],
                            xtv[eb][:, jb * P : (jb + 1) * P],
                            wvt[eb][:, coff : coff + clen],
                            start=(eb == 0),
                            stop=(eb == NE - 1),
                        )
                    nc.vector.tensor_add(
                        vv[jb][:, coff : coff + clen],
                        pv[:],
                        bv_sb[:, coff : coff + clen],
                    )

            wkt = load_w(wk, D, "wk")
            xtk = transposed_input(xk, "xk")
            for hb in range(NE):
                for ic in range(NIC):
                    pq = psum_big.tile([P, IC], F32, tag="big", name="pq")
                    for eb in range(NE):
                        nc.tensor.matmul(
                            pq[:],
                            wkt[eb][:, hb * P : (hb + 1) * P],
                            xtk[eb][:, ic * IC : (ic + 1) * IC],
                            start=(eb == 0),
                            stop=(eb == NE - 1),
                        )
                    nc.vector.tensor_scalar_add(
                        kT[hb][:, ic * IC : (ic + 1) * IC],
                        pq[:],
                        bk_sb[:, hb : hb + 1],
                    )

            # q-projection resources: prefetch during phase A
            wqt = load_w(wq, D, "wq")
            xtq = transposed_input(xq, "xq")

        # ---- phases B+C: attention + q-proj + output projection ---------
        with (
            tc.tile_pool(name="psum_sc", bufs=3, space="PSUM") as psum_sc,
            tc.tile_pool(name="psum_ov", bufs=2, space="PSUM") as psum_ov,
        ):
            def emit_qproj(hb):
                for ic in range(NIC):
                    pq2 = psum_sc.tile([P, 2 * IC], F32, tag="sc", name="pq2")
                    pq = pq2[:, 0:IC]
                    for eb in range(NE):
                        nc.tensor.matmul(
                            pq,
                            wqt[eb][:, hb * P : (hb + 1) * P],
                            xtq[eb][:, ic * IC : (ic + 1) * IC],
                            start=(eb == 0),
                            stop=(eb == NE - 1),
                        )
                    nc.vector.tensor_scalar_add(
                        qT[hb][:, ic * IC : (ic + 1) * IC],
                        pq,
                        bq_sb[:, hb : hb + 1],
                    )

            rounds = [(ic, t) for ic in range(NIC) for t in range(NE)]
            et_store = {}

            def emit_scores(r):
                ic, t = rounds[r]
                icsl = slice(ic * IC, (ic + 1) * IC)
                et = {}
                for jb in range(NB):
                    ps = psum_sc.tile([P, 2 * IC], F32, tag="sc", name="ps")
                    for hh in range(2):
                        off = 64 * hh
                        nc.tensor.matmul(
                            ps[:, hh * IC : (hh + 1) * IC],
                            kT[t][off : off + DH, jb * P : (jb + 1) * P],
                            qT[t][off : off + DH, icsl],
                            start=True,
                            stop=True,
                            tile_position=(off, 0),
                        )
                    e = etp.tile([P, 2 * IC], F16, tag="et", name="et")
                    nc.scalar.activation(e[:], ps[:], Exp, scale=float(SCALE))
                    et[jb] = e
                et_store[r] = et

            def emit_attnv(r):
                ic, t = rounds[r]
                icsl = slice(ic * IC, (ic + 1) * IC)
                et = et_store.pop(r)
                for hh in range(2):
                    h = 2 * t + hh
                    po = psum_ov.tile([DH + 1, IC], F32, tag="ov", name="po")
                    for jb in range(NB):
                        nc.tensor.matmul(
                            po[:],
                            vv[jb][:, h * (DH + 1) : (h + 1) * (DH + 1)],
                            et[jb][:, hh * IC : (hh + 1) * IC],
                            start=(jb == 0),
                            stop=(jb == NB - 1),
                        )
                    # copy PSUM out early (incl. denominator row) so the
                    # 2-bank ov pool isn't held through the recip chain
                    ou = oup.tile([DH + 1, IC], F32, tag="ou", name="ou")
                    nc.vector.tensor_copy(ou[:], po[:])
                    d0 = dnp.tile([1, IC], F32, tag="d0", name="d0")
                    nc.sync.dma_start(d0[0:1, :], ou[DH : DH + 1, :])
                    dn = rbp.tile([DH, IC], F32, tag="dn", name="dn")
                    nc.gpsimd.partition_broadcast(
                        dn[:], d0[0:1, :], channels=DH
                    )
                    rb = rbp.tile([DH, IC], F32, tag="rb", name="rb")
                    nc.vector.reciprocal_approx_fast(rb[:], dn[:])
                    if hh == 0:
                        nc.vector.tensor_mul(
                            OT[t][0:DH, icsl], ou[0:DH, :], rb[:]
                        )
                    else:
                        sg = stp.tile([DH, IC], F16, tag="sg", name="sg")
                        nc.vector.tensor_mul(sg[:], ou[0:DH, :], rb[:])
                        nc.sync.dma_start(OT[t][DH:P, icsl], sg[:])

            nr = len(rounds)
            # q-projection interleaved ahead of the score rounds that use it
            emit_qproj(0)
            emit_qproj(1)
            emit_scores(0)
            for r in range(NE - 2):
                emit_qproj(r + 2)
                emit_scores(r + 1)
                emit_attnv(r)
            # q-projection fully emitted -> free its pools, load Wo
            stA.close()
            with tc.tile_pool(name="wop", bufs=8) as wop:
                wot = []
                for hb in range(NE):
                    wo_sb = wop.tile([P, D], F16, tag="wo", name=f"wo{hb}")
                    nc.sync.dma_start(wo_sb[:], wo[hb * P : (hb + 1) * P, :])
                    wot.append(wo_sb)

                def emit_outproj(ib, ec):
                    pf2 = psum_sc.tile([P, 2 * IC], F32, tag="sc", name="pf2")
                    pf = pf2[:, 0:IC]
                    for hb in range(NE):
                        nc.tensor.matmul(
                            pf,
                            OT[hb][:, ib * P : (ib + 1) * P],
                            wot[hb][:, ec * IC : (ec + 1) * IC],
                            start=(hb == 0),
                            stop=(hb == NE - 1),
                        )
                    fo = fop.tile([P, IC], F32, tag="fo", name="fo")
                    nc.vector.tensor_add(
                        fo[:], pf[:], bo_sb[:, ec * IC : (ec + 1) * IC]
                    )
                    nc.sync.dma_start(
                        out[ib * P : (ib + 1) * P, ec * IC : (ec + 1) * IC],
                        fo[:],
                    )

                fillers = [
                    (ib, ec) for ib in range(NB // 2) for ec in range(NIC)
                ]
                late = [
                    (ib, ec) for ib in range(NB // 2, NB) for ec in range(NIC)
                ]
                for r in range(NE - 2, nr):
                    if r + 1 < nr:
                        emit_scores(r + 1)
                    emit_attnv(r)
                    if r >= NE and fillers:
                        emit_outproj(*fillers.pop(0))
                for ib, ec in fillers + late:
                    emit_outproj(ib, ec)

    nc.compile()
    return nc


_NC = None


def _get_nc():
    global _NC
    if _NC is None:
        _NC = _build()
    return _NC


def _prep_inputs(query, key, value, Wq, bq, Wk, bk, Wv, bv, Wo, bo):
    f32 = lambda a: np.ascontiguousarray(np.asarray(a, dtype=np.float32))
    f16 = lambda a: np.ascontiguousarray(
        np.asarray(a, dtype=np.float32).astype(np.float16)
    )
    query, key, value = f16(query), f16(key), f16(value)
    bq, bk, bv, bo = f32(bq), f32(bk), f32(bv), f32(bo)
    Wv = f32(Wv)

    wva = np.zeros((D, VW), dtype=np.float32)
    bva = np.zeros((VW,), dtype=np.float32)
    for h in range(H):
        wva[:, h * (DH + 1) : h * (DH + 1) + DH] = Wv[:, h * DH : (h + 1) * DH]
        bva[h * (DH + 1) : h * (DH + 1) + DH] = bv[h * DH : (h + 1) * DH]
        bva[h * (DH + 1) + DH] = 1.0
    common = {
        "wq": f16(Wq),
        "wk": f16(Wk),
        "wva": wva.astype(np.float16),
        "wo": f16(Wo),
        "bq2": np.ascontiguousarray(bq.reshape(NE, P).T),
        "bk2": np.ascontiguousarray(bk.reshape(NE, P).T),
        "bvb": np.ascontiguousarray(np.tile(bva, (P, 1))),
        "bob": np.ascontiguousarray(np.tile(bo, (P, 1))),
    }
    in_maps = [
        dict(common, xq=query[c], xk=key[c], xv=value[c]) for c in range(B)
    ]
    return in_maps


def kernel(query, key, value, Wq, bq, Wk, bk, Wv, bv, Wo, bo):
    nc = _get_nc()
    in_maps = _prep_inputs(query, key, value, Wq, bq, Wk, bk, Wv, bv, Wo, bo)
    res = run_bass_kernel_spmd(nc, in_maps, core_ids=list(range(B)))
    return np.stack([res.results[c]["out"] for c in range(B)], axis=0)
